# revision 65
# baseline (speedup 1.0000x reference)
"""Trainium2 Bass kernel for nn_MAPLoss (smooth-AP loss, N=512, D=256, K=0.001).

v7 (43.5us -> ~25.5us vs the v5 baseline). The loss reads prec[i] only at
positive (query, item) pairs (~3600 of 512*511), so each core evaluates just
its ~450 pairs, bin-packed row-atomically into [128 x 512] blocks.

Data-parallel over the 8 cores: each core gets a full permuted copy of the
inputs with its 64 rows first (the "all-gather" is free since the host
replicates), computes its pair block, and returns per-block partial sums
that the host adds and finishes as 1 - num/cnt.

Performance structure (what made it fast):
  - q^T ships as fp8_e4m3 (128KB), one-hot metadata as bf16, iota generated
    on device; DMAs are partition-split across the sync+scalar queues
    (per-queue DMA is packet-bound at ~45ns per partition-row).
  - norms: DVE square + ones-matmul whose output partitions are free, so
    sumsq lands pre-broadcast on 64 partitions; Sqrt and then
    reciprocal_approx_fast run on [64,512] at the same per-lane cost as
    [1,512]; row invs are a [64,64] diag-extract. No broadcast matmul, no
    transpose, no SBUF round trips.
  - R = -1000 * cos-sim in bf16 (the -1/K sigmoid scale is folded into the
    row invs), so the per-block fused (iota==sel)*rrep accumulation yields
    the sigmoid BIAS directly and den = sigmoid(-rrep + bias) needs no
    extra DVE op. Replication matmuls are bf16 (1 cycle/row).
  - the pair-pair sigmoid is a step to 5e-5 at K=0.001, so acc is a DVE
    is_lt count against the bdgs-gathered biases (no ACT, no gpsimd hop),
    and the self-column sigmoid(1000(1-rg)) == 1.0 to fp32 is a constant.
  - exactly three ACT_TABLE_LOADs, all off the critical path (a dummy
    sigmoid depending on the Sqrt output pins the load order).
  - per-block den_adj/reciprocal/prec*w run in DVE gaps so the tail after
    the last block is just count -> reduce-matmul -> copy -> DMA.
Remaining time is ~10us of fixed BSP preamble/postamble (barriers,
engine register loads, drains), ~3.5us DMA latency, ~5us norm+R setup
chain, ~4us ACT-bound block phase, ~2us tail.
All float FLOPs run on device; the host only derives integer metadata
(permutation, pair slots, one-hot selectors, 1/npos weights) from
`target` and casts dtypes.
"""

import numpy as np
from contextlib import ExitStack

N = 512
D = 256
NCORES = 8
RPC = N // NCORES   # rows per core = 64
SLOTS = 16          # max positives per row (max npos observed is 13)
KINV = 1000.0       # 1/K


def _build_program(nblk):
    import concourse.bacc as bacc
    import concourse.tile as tile
    import concourse.mybir as mybir

    fp32 = mybir.dt.float32
    bf16 = mybir.dt.bfloat16
    fp8 = mybir.dt.float8e4
    ALU = mybir.AluOpType
    ACT = mybir.ActivationFunctionType

    NDC = D // 128          # 2 dim chunks of qT
    BDGS0 = 0               # meta16 column offsets
    IBS0 = 128 * nblk
    MG0 = 144 * nblk
    M16 = 160 * nblk

    nc = bacc.Bacc("TRN2", target_bir_lowering=False, debug=False,
                   num_devices=NCORES)
    qt2_dram = nc.dram_tensor("qt2", [128, NDC * N], fp8,
                              kind="ExternalInput").ap()
    rep_dram = nc.dram_tensor("rep", [RPC, 128 * nblk], bf16,
                              kind="ExternalInput").ap()
    m16_dram = nc.dram_tensor("m16", [128, M16], bf16,
                              kind="ExternalInput").ap()
    m32_dram = nc.dram_tensor("m32", [128, 2 * nblk], fp32,
                              kind="ExternalInput").ap()
    out_dram = nc.dram_tensor("out", [nblk, 1], fp32, kind="ExternalOutput").ap()

    with tile.TileContext(nc) as tc, ExitStack() as ctx:
        const = ctx.enter_context(tc.tile_pool(name="const", bufs=1))
        persist = ctx.enter_context(tc.tile_pool(name="persist", bufs=1))
        setup_ctx = ctx.enter_context(ExitStack())
        spsum = setup_ctx.enter_context(
            tc.tile_pool(name="spsum", bufs=1, space="PSUM"))
        ssb = setup_ctx.enter_context(tc.tile_pool(name="ssb", bufs=1))

        # --- constants (gpsimd) + the input DMAs on separate rings ---
        ones_st = const.tile([128, RPC], bf16, tag="ones_st")
        nc.gpsimd.memset(ones_st[:], 1.0)
        ones_red = const.tile([128, 1], fp32, tag="ones_red")
        nc.gpsimd.memset(ones_red[:], 1.0)
        iota_f = const.tile([128, N], fp32, tag="iota_f")
        nc.gpsimd.iota(iota_f[:], pattern=[[1, N]], base=0,
                       channel_multiplier=0,
                       allow_small_or_imprecise_dtypes=True)
        pidx = const.tile([128, 1], fp32, tag="pidx")
        nc.gpsimd.iota(pidx[:], pattern=[[1, 1]], base=0,
                       channel_multiplier=1,
                       allow_small_or_imprecise_dtypes=True)

        # DMA throughput is packet-bound (~45ns per partition-row packet
        # regardless of 1KB vs 2KB), so keep full 2KB rows and split by
        # partition range across two queues.
        qt2 = persist.tile([128, NDC * N], fp8, tag="qt2")
        nc.sync.dma_start(qt2[0:64, :], qt2_dram[0:64, :])
        nc.scalar.dma_start(qt2[64:128, :], qt2_dram[64:128, :])
        m16 = persist.tile([128, M16], bf16, tag="m16")
        nc.sync.dma_start(m16[0:64, :], m16_dram[0:64, :])
        nc.scalar.dma_start(m16[64:128, :], m16_dram[64:128, :])
        m32 = persist.tile([128, 2 * nblk], fp32, tag="m32")
        nc.gpsimd.dma_start(m32[:], m32_dram)
        rep = persist.tile([RPC, 128 * nblk], bf16, tag="rep")
        nc.gpsimd.dma_start(rep[:], rep_dram)

        # --- norms. The ones-matmul's cost scales with moving columns, not
        # output partitions, so broadcast sumsq to RPC partitions for free;
        # Sqrt/reciprocal on [RPC, N] cost the same as on [1, N] (per-lane).
        qtsq = ssb.tile([128, NDC * N], bf16, tag="qtsq")
        ss_ps = spsum.tile([RPC, N], fp32, tag="ss_ps")
        g_ps = spsum.tile([RPC, N], fp32, tag="g_ps")
        for c in range(NDC):
            nc.vector.tensor_mul(qtsq[:, N * c:N * (c + 1)],
                                 qt2[:, N * c:N * (c + 1)],
                                 qt2[:, N * c:N * (c + 1)])
            nc.tensor.matmul(ss_ps[:], ones_st[:],
                             qtsq[:, N * c:N * (c + 1)],
                             start=(c == 0), stop=(c == NDC - 1))
            # Gram for rows 0..RPC-1 (PE, interleaved with the norm matmuls)
            nc.tensor.matmul(g_ps[:], qt2[:, N * c:N * c + RPC],
                             qt2[:, N * c:N * (c + 1)],
                             start=(c == 0), stop=(c == NDC - 1))

        # G to SBUF on the (idle) DVE so the R STT has only one PSUM operand
        g_sb = ssb.tile([RPC, N], bf16, tag="g_sb")
        nc.vector.tensor_copy(g_sb[:], g_ps[:])
        # row sumsqs = diag of the broadcast ss_ps, extracted on the DVE
        # during the Sqrt window (ss_ps[p, j] = sumsq_j for every p)
        sumsq0 = ssb.tile([RPC, 1], fp32, tag="sumsq0")
        dscr = ssb.tile([RPC, RPC], bf16, tag="dscr")
        nc.vector.scalar_tensor_tensor(dscr[:], iota_f[0:RPC, 0:RPC],
                                       pidx[0:RPC, :], ss_ps[:, 0:RPC],
                                       op0=ALU.is_equal, op1=ALU.mult,
                                       accum_out=sumsq0[:])

        norm_b = ssb.tile([RPC, N], fp32, tag="norm_b")
        nc.scalar.activation(norm_b[:], ss_ps[:], ACT.Sqrt)
        norm0 = ssb.tile([RPC, 1], fp32, tag="norm0")
        nc.scalar.activation(norm0[:], sumsq0[:], ACT.Sqrt)
        # warm the sigmoid act table right after the last sqrt-table use;
        # reads norm_b so the scheduler cannot hoist it before the Sqrts
        dummy = ssb.tile([1, 1], fp32, tag="dummy")
        nc.scalar.activation(dummy[:], norm_b[0:1, 0:1], ACT.Sigmoid)
        invb = ssb.tile([RPC, N], fp32, tag="invb")
        nc.vector.reciprocal_approx_fast(invb[:], norm_b[:])
        # row invs scaled by -1000 (the -1/K sigmoid scale): R then holds
        # -1000 * cos-sim, so gtmp's accumulator IS the sigmoid bias
        inv0 = ssb.tile([RPC, 1], fp32, tag="inv0")
        nc.vector.reciprocal_approx_fast(inv0[:], norm0[:])
        inv0m = ssb.tile([RPC, 1], fp32, tag="inv0m")
        nc.vector.tensor_scalar_mul(inv0m[:], inv0[:], -KINV)

        # R = diag(-1000*inv) G diag(inv), stored bf16 for the replication.
        # Built in column halves so block 0's replication matmul can start
        # on the first half while the DVE finishes the second.
        HN = N // 2
        R = persist.tile([RPC, N], bf16, tag="R")
        for h in range(2):
            nc.vector.scalar_tensor_tensor(
                R[:, HN * h:HN * (h + 1)], g_sb[:, HN * h:HN * (h + 1)],
                inv0m[:], invb[:, HN * h:HN * (h + 1)],
                op0=ALU.mult, op1=ALU.mult)

        # --- main: one [128, N] block per pair-bin ---
        bias_flat = persist.tile([128, nblk], fp32, tag="bias_flat")
        den_flat = persist.tile([128, nblk], fp32, tag="den_flat")
        acc_flat = persist.tile([128, nblk], fp32, tag="acc_flat")
        den_adj = persist.tile([128, nblk], fp32, tag="den_adj")
        recip_f = persist.tile([128, nblk], fp32, tag="recip_f")
        wrecip = persist.tile([128, nblk], fp32, tag="wrecip")
        setup_ctx.close()
        s_pool = ctx.enter_context(tc.tile_pool(name="s", bufs=3))
        rp_pool = ctx.enter_context(tc.tile_pool(name="rp", bufs=1, space="PSUM"))
        gp_pool = ctx.enter_context(tc.tile_pool(name="gp", bufs=1, space="PSUM"))
        g2_all = gp_pool.tile([128, SLOTS * nblk], fp32, tag="g2_all")

        # all replication matmuls first: PE runs ahead so the DVE/ACT block
        # chains never wait on it
        rreps = []
        for b in range(nblk):
            rrep = rp_pool.tile([128, N], fp32, tag=f"rrep{b}", name=f"rrep{b}")
            if b == 0:
                # column-halved so it can chase the R halves off the DVE
                for h in range(2):
                    nc.tensor.matmul(rrep[:, HN * h:HN * (h + 1)],
                                     rep[:, 0:128],
                                     R[:, HN * h:HN * (h + 1)],
                                     start=True, stop=True)
            else:
                nc.tensor.matmul(rrep[:], rep[:, 128 * b:128 * (b + 1)],
                                 R[:], start=True, stop=True)
            rreps.append(rrep)
        for b in range(nblk):
            rrep = rreps[b]
            # bias[p] = -1000*R[row(p), sel(p)] via fused iota==sel
            # multiply-accumulate (rrep already carries the -1000 scale)
            tmp = s_pool.tile([128, N], bf16, tag="gtmp")
            nc.vector.scalar_tensor_tensor(
                tmp[:], iota_f[:], m32[:, b:b + 1], rrep[:],
                op0=ALU.is_equal, op1=ALU.mult,
                accum_out=bias_flat[:, b:b + 1])
            sp = s_pool.tile([128, N], bf16, tag="sp")
            nc.scalar.activation(sp[:], rrep[:], ACT.Sigmoid,
                                 bias=bias_flat[:, b:b + 1], scale=-1.0,
                                 accum_out=den_flat[:, b:b + 1])
            # acc from positive-positive pairs: gather bias values of the
            # same row's slots with a block-diagonal selector matmul. With
            # K=0.001 the pair-pair sigmoid is a step to within 5e-5 except
            # at near-ties, so count rg_s' > rg_p directly on the DVE:
            # g2 < bias  <=>  -1000*rg_s' < -1000*rg_p  <=>  rg_s' > rg_p.
            rh = s_pool.tile([128, SLOTS], bf16, tag="rh")
            nc.gpsimd.tensor_scalar(rh[:], m16[:, IBS0 + SLOTS * b:IBS0 + SLOTS * (b + 1)],
                                    bias_flat[:, b:b + 1], None, op0=ALU.mult)
            nc.tensor.matmul(g2_all[:, SLOTS * b:SLOTS * (b + 1)],
                             m16[:, BDGS0 + 128 * b:BDGS0 + 128 * (b + 1)],
                             rh[:], start=True, stop=True)
            # den_adj + w/den for this block while the DVE has slack
            nc.vector.tensor_scalar_add(den_adj[:, b:b + 1],
                                        den_flat[:, b:b + 1], -0.5)
            nc.vector.reciprocal_approx_fast(recip_f[:, b:b + 1],
                                             den_adj[:, b:b + 1])
            nc.vector.tensor_mul(wrecip[:, b:b + 1], recip_f[:, b:b + 1],
                                 m32[:, nblk + b:nblk + b + 1])
        # pair-pair step-counts + per-block weighted precision after the
        # gtmp stream; only the last block's short chain sits on the tail.
        # pw = (acc + 1.0) * (w/den): the +1.0 is the reference's +1 minus
        # the own-slot 0.5 the step-count misses; w/den was precomputed.
        ep = ctx.enter_context(tc.tile_pool(name="ep", bufs=1))
        pw = ep.tile([128, nblk], fp32, tag="pw")
        for b in range(nblk):
            sacc = s_pool.tile([128, SLOTS], bf16, tag="sacc")
            nc.vector.scalar_tensor_tensor(
                sacc[:], g2_all[:, SLOTS * b:SLOTS * (b + 1)],
                bias_flat[:, b:b + 1],
                m16[:, MG0 + SLOTS * b:MG0 + SLOTS * (b + 1)],
                op0=ALU.is_lt, op1=ALU.mult,
                accum_out=acc_flat[:, b:b + 1])
            nc.vector.tensor_scalar(pw[:, b:b + 1], acc_flat[:, b:b + 1],
                                    1.0, wrecip[:, b:b + 1],
                                    op0=ALU.add, op1=ALU.mult)
        red = gp_pool.tile([nblk, 1], fp32, tag="red", bufs=1)
        nc.tensor.matmul(red[:], pw[:], ones_red[:], start=True, stop=True)
        out_sb = ep.tile([nblk, 1], fp32, tag="out_sb")
        nc.vector.tensor_copy(out_sb[:], red[:])
        nc.sync.dma_start(out_dram, out_sb[:])

    nc.compile()
    return nc


def make_in_maps(query: np.ndarray, target: np.ndarray):
    """Host-side sharding + pair-packing metadata (per-core rolled copies)."""
    import ml_dtypes

    query = np.ascontiguousarray(np.asarray(query), dtype=np.float32)
    tgt = np.asarray(target).reshape(-1)

    # balance rows across cores by positive-pair count (any assignment is
    # valid: each core sees a full permuted copy with its rows first)
    npos_all = np.array([np.sum(tgt == tgt[i]) - 1 for i in range(N)])
    ncnt = int(np.sum(npos_all > 0))
    loads = [0] * NCORES
    assign = [[] for _ in range(NCORES)]
    for i in sorted(range(N), key=lambda i: -npos_all[i]):
        cands = [c for c in range(NCORES) if len(assign[c]) < RPC]
        c = min(cands, key=lambda c: loads[c])
        assign[c].append(i)
        loads[c] += int(npos_all[i])

    cores = []
    for c in range(NCORES):
        mine = assign[c]
        others = [i for i in range(N) if i not in set(mine)]
        perm = np.array(mine + others)
        t_r = tgt[perm]
        rows = []  # per row: positive indices (in permuted coords)
        for q in range(RPC):
            pos = np.flatnonzero(t_r == t_r[q])
            pos = pos[pos != q]
            assert len(pos) <= SLOTS, f"npos {len(pos)} > SLOTS {SLOTS}"
            rows.append(pos)
        # bin-pack rows (row-atomic, best-fit decreasing) into <=128-pair bins
        blocks = []
        fill = []
        order = sorted((q for q in range(RPC) if len(rows[q]) > 0),
                       key=lambda q: -len(rows[q]))
        for q in order:
            npos = len(rows[q])
            best = -1
            for i, f in enumerate(fill):
                if f + npos <= 128 and (best < 0 or f > fill[best]):
                    best = i
            if best < 0:
                blocks.append([q])
                fill.append(npos)
            else:
                blocks[best].append(q)
                fill[best] += npos
        cores.append((perm, rows, blocks))
    nblk = max(len(b) for _, _, b in cores)

    in_maps = []
    for perm, rows, blocks in cores:
        q_r = query[perm]                      # [N, D]
        qt = q_r.T                             # [D, N]
        qt2 = np.ascontiguousarray(
            qt.reshape(D // 128, 128, N).transpose(1, 0, 2).reshape(128, -1)
        ).astype(ml_dtypes.float8_e4m3)

        M16 = 160 * nblk
        m16 = np.zeros((128, M16), dtype=np.float32)
        repm = np.zeros((RPC, 128 * nblk), dtype=np.float32)
        m32 = np.zeros((128, 2 * nblk), dtype=np.float32)
        m32[:, 0:nblk] = -1.0                  # sel default: matches no iota
        BDGS0, IBS0, MG0 = 0, 128 * nblk, 144 * nblk
        for b, rowlist in enumerate(blocks):
            p = 0
            for q in rowlist:
                npos = len(rows[q])
                pr = range(p, p + npos)
                for s, j in enumerate(rows[q]):
                    m32[p + s, b] = float(j)                 # sel
                    m32[p + s, nblk + b] = 1.0 / npos        # w
                    m16[p + s, IBS0 + SLOTS * b + s] = 1.0   # ibs
                    m16[p + s, MG0 + SLOTS * b:MG0 + SLOTS * b + npos] = 1.0
                for k in pr:
                    for p2 in pr:
                        m16[k, BDGS0 + 128 * b + p2] = 1.0   # bdgs
                    repm[q, 128 * b + k] = 1.0               # rep
                p += npos
        in_maps.append({
            "qt2": qt2,
            "rep": repm.astype(ml_dtypes.bfloat16),
            "m16": m16.astype(ml_dtypes.bfloat16),
            "m32": m32,
        })
    return in_maps, nblk, ncnt


_NC_CACHE = {}


def kernel(query: np.ndarray, target: np.ndarray) -> np.ndarray:
    from concourse import bass_utils

    in_maps, nblk, ncnt = make_in_maps(query, target)
    global _NC_CACHE
    if nblk not in _NC_CACHE:
        _NC_CACHE[nblk] = _build_program(nblk)
    nc = _NC_CACHE[nblk]

    res = bass_utils.run_bass_kernel_spmd(nc, in_maps, core_ids=list(range(NCORES)))
    num = 0.0
    for c in range(NCORES):
        num += float(np.sum(res.results[c]["out"]))
    mean_ap = num / max(float(ncnt), 1.0)
    return np.float32(1.0 - mean_ap)
